# revision 1
# baseline (speedup 1.0000x reference)
"""Causal self-attention (B=2, T=2048, C=1024, H=16) on 8 trn2 NeuronCores.

Sharding: core c handles batch b=c//4 and head group g=c%4 (4 heads each).
Data parallel on B, tensor parallel on H; W_attn/W_proj sliced per head
group; host sums the 4 tensor-parallel partial projection outputs per batch.

v2 layout notes (v1 ran the attention phase at the PE's mid p-state --
~0.83 ns/col instead of 0.42 -- because exp-wait bubbles reset the HAM
ramp; everything here is about keeping the PE stream gapless):
  - x pre-transposed from host as xT [C, T], DMA'd as [128, 512] chunks in
    consumption order so the first qkT matmul starts ~6us in.
  - qkT [512, T] = W_qk.T @ x.T accumulated over 8 c-tiles; psum->sbuf
    bias-copies on DVE (tensor_scalar_add), keeping ACT exp-only.
  - v natural [T, 256] with the whole v tile memset to 1.0 first: cols
    0..63 of each per-head 128-col block stay ones, so the PV matmul
    replicates the softmax denominator l across psum partitions 0..63 and
    reciprocal_approx_fast runs directly on [64, N] -- no GpSimd
    partition_broadcast on the critical path.
  - attention runs pair p=0 (queries 0..1023) first, then p=1; S^T per
    (head, j-tile 128) into [128, 1024] psum via 512-wide matmuls from the
    exact causal column (no widening; bf16 is 1 cyc/row at any width);
    the mixed diagonal block gets -8192 accumulated via a lower-triangular
    matmul so exp yields exact zeros there.
  - filler matmuls are interleaved into the attention block stream so the
    PE always has independent ready work while ACT runs exp: v tiles 2..7
    during p0/h0, v tiles 8..15 during p0/h1-h3, projection of pair 0
    during p1. Tail = projection of pair 1 with psum->sbuf copies split
    between ACT and DVE.
  - output is bf16 [T, C]; host accumulates the 4 tensor-parallel partials
    in fp32 and adds b_proj.
All matmul operands are bf16 (fp32 accumulate in PSUM).
"""

import os
import numpy as np
import ml_dtypes

import concourse.bacc as bacc
import concourse.mybir as mybir
import concourse.tile as tile
from concourse.bass_utils import run_bass_kernel_spmd
from concourse.masks import make_upper_triangular

B, T, C, H = 2, 2048, 1024, 16
D = C // H          # 64
HPC = H // 4        # 4 heads per core
QK = 2 * HPC * D    # 512 rows of qkT (q then k)
V = HPC * D         # 256 v columns
F32 = mybir.dt.float32
BF16 = mybir.dt.bfloat16
PAIR = 1024         # queries per attention pass (2 psum banks)
AF = mybir.ActivationFunctionType
NC_ = C // 128      # 8 c-tiles
TCH = 4             # 512-col xT chunks per c-tile

_cache = {}


def _build():
    nc = bacc.Bacc("TRN2", target_bir_lowering=False, debug=False, num_devices=8)
    xT = nc.dram_tensor("xT", [C, T], BF16, kind="ExternalInput").ap()
    w_qk = nc.dram_tensor("w_qk", [C, QK], BF16, kind="ExternalInput").ap()
    b_qk = nc.dram_tensor("b_qk", [QK, 1], F32, kind="ExternalInput").ap()
    w_v = nc.dram_tensor("w_v", [C, V], BF16, kind="ExternalInput").ap()
    b_v = nc.dram_tensor("b_v", [1, V], F32, kind="ExternalInput").ap()
    w_pr = nc.dram_tensor("w_pr", [V, C], BF16, kind="ExternalInput").ap()
    out = nc.dram_tensor("out", [T, C], BF16, kind="ExternalOutput").ap()

    with tile.TileContext(nc) as tc:
        with (
            tc.tile_pool(name="const", bufs=1) as cpool,
            tc.tile_pool(name="xt", bufs=1) as xpool,
            tc.tile_pool(name="w", bufs=1) as wpool,
            tc.tile_pool(name="qk", bufs=1) as qkpool,
            tc.tile_pool(name="vaug", bufs=1) as vpool,
        ):
            # ---- constants ----
            # tri01[j, i] = 1 where i >= j: multiplicative causal mask for
            # the mixed diagonal 128x128 block (applied on DVE after exp;
            # the PV skew gives the mask a full block of slack)
            tri01 = cpool.tile([128, 128], BF16, name="tri01")
            make_upper_triangular(nc, tri01[:], val=1.0, diag=True)
            # preload the ACT exp table during the DMA lead-in
            warm = cpool.tile([1, 16], BF16, name="warm")
            nc.scalar.activation(warm[:], tri01[0:1, 0:16], AF.Exp)

            # v tiles: whole tile preset to 1.0 (gpsimd, off critical path);
            # cols 64..127 of each head block overwritten with v rows later.
            v_t = [vpool.tile([128, HPC, 128], BF16, name=f"v{t}")
                   for t in range(T // 128)]
            for t in range(T // 128):
                nc.gpsimd.memset(v_t[t][:], 1.0)

            # ---- input DMA in consumption order (biases first: tiny) ----
            bqk_t = []
            for j in range(QK // 128):
                bt = cpool.tile([128, 1], F32, name=f"bqk{j}")
                nc.sync.dma_start(bt[:], b_qk[j * 128:(j + 1) * 128, :])
                bqk_t.append(bt)
            bv_row = cpool.tile([1, V], F32, name="bv_row")
            nc.sync.dma_start(bv_row[:], b_v[:])
            bv_full = cpool.tile([128, V], F32, name="bv_full")
            nc.gpsimd.partition_broadcast(bv_full[:], bv_row[:])
            # wqk as j-quarters and the first xT wave halved so the first
            # qkT group's inputs (768KB) land ~7us in instead of ~12
            wqk_t = []
            xtc = [[None] * TCH for _ in range(NC_)]
            for c in range(NC_):
                w = wpool.tile([128, QK], BF16, name=f"wqk{c}")
                nc.sync.dma_start(w[:, 0:128], w_qk[c * 128:(c + 1) * 128, 0:128])
                wqk_t.append(w)
                t = xpool.tile([128, 512], BF16, name=f"xt{c}_0")
                nc.sync.dma_start(t[:, 0:256], xT[c * 128:(c + 1) * 128, 0:256])
                xtc[c][0] = t
            for c in range(NC_):
                nc.sync.dma_start(wqk_t[c][:, 128:256],
                                  w_qk[c * 128:(c + 1) * 128, 128:256])
                nc.sync.dma_start(xtc[c][0][:, 256:512],
                                  xT[c * 128:(c + 1) * 128, 256:512])
            for c in range(NC_):
                nc.sync.dma_start(wqk_t[c][:, 256:512],
                                  w_qk[c * 128:(c + 1) * 128, 256:512])
            for c in range(NC_):
                t = xpool.tile([128, 512], BF16, name=f"xt{c}_1")
                nc.sync.dma_start(t[:], xT[c * 128:(c + 1) * 128, 512:1024])
                xtc[c][1] = t
            wv_t = []
            for c in range(NC_):
                t = wpool.tile([128, V], BF16, name=f"wv{c}")
                nc.sync.dma_start(t[:], w_v[c * 128:(c + 1) * 128, :])
                wv_t.append(t)
            for c in range(NC_):
                t = xpool.tile([128, 512], BF16, name=f"xt{c}_2")
                nc.sync.dma_start(t[:], xT[c * 128:(c + 1) * 128, 1024:1536])
                xtc[c][2] = t
            wpr_t = []
            for k in range(V // 128):
                t = wpool.tile([128, C], BF16, name=f"wpr{k}")
                nc.sync.dma_start(t[:], w_pr[k * 128:(k + 1) * 128, :])
                wpr_t.append(t)
            for c in range(NC_):
                t = xpool.tile([128, 512], BF16, name=f"xt{c}_3")
                nc.sync.dma_start(t[:], xT[c * 128:(c + 1) * 128, 1536:2048])
                xtc[c][3] = t

            qk_t = [qkpool.tile([128, T], BF16, name=f"qk{j}")
                    for j in range(QK // 128)]

            def make_v(tt, tag, bufs):
                """Produce v_t[tt] (8 accumulating matmuls + DVE bias add)."""
                def go(pool):
                    ps = pool.tile([128, V], F32, name="v_ps",
                                   tag=tag, bufs=bufs)
                    ch, sub = tt // 4, tt % 4
                    for c in range(NC_):
                        nc.tensor.matmul(
                            ps[:],
                            xtc[c][ch][:, sub * 128:(sub + 1) * 128],
                            wv_t[c][:],
                            start=(c == 0), stop=(c == NC_ - 1))
                    nc.vector.tensor_add(
                        v_t[tt][:, :, 64:64 + D],
                        ps[:].rearrange("p (h d) -> p h d", h=HPC),
                        bv_full[:].rearrange("p (h d) -> p h d", h=HPC))
                return go

            # ---- qkT: 16 psum groups, tch-major (matches DMA arrival
            # waves); bias-copy to sbuf on DVE ----
            with tc.tile_pool(name="ps1", bufs=1, space="PSUM") as ps1:
                def qk_group(j, tch):
                    ps = ps1.tile([128, 512], F32, name="qk_ps",
                                  tag="qk_ps", bufs=8)
                    for c in range(NC_):
                        nc.tensor.matmul(
                            ps[:],
                            wqk_t[c][:, j * 128:(j + 1) * 128],
                            xtc[c][tch][:],
                            start=(c == 0), stop=(c == NC_ - 1))
                    nc.vector.tensor_scalar_add(
                        qk_t[j][:, tch * 512:(tch + 1) * 512],
                        ps[:], bqk_t[j][:])

                for tch in range(TCH):
                    for j in range(QK // 128):
                        qk_group(j, tch)
                # first two v tiles before attention starts (share the ring)
                make_v(0, "qk_ps", 8)(ps1)
                make_v(1, "qk_ps", 8)(ps1)

            # ================= attention + projection =================
            with (
                tc.tile_pool(name="att_sb", bufs=1) as apool,
                tc.tile_pool(name="osb", bufs=1) as opool,
                tc.tile_pool(name="ps2", bufs=1, space="PSUM") as ps2,
            ):
                osb = {}
                yn_of = {}

                def proj_mm(p, tt, cc, copy_engine, tag, bufs):
                    """One projection psum group (2 matmuls + copy + DMA)."""
                    i0 = p * PAIR
                    def go(pool):
                        o_ps = pool.tile([128, 512], F32, name="o_ps",
                                         tag=tag, bufs=bufs)
                        for k in range(V // 128):
                            nc.tensor.matmul(
                                o_ps[:],
                                yn_of[p][k][:, tt * 128:(tt + 1) * 128],
                                wpr_t[k][:, cc * 512:(cc + 1) * 512],
                                start=(k == 0), stop=(k == V // 128 - 1))
                        if cc == 0:
                            osb[(p, tt)] = opool.tile(
                                [128, C], BF16, name="osb",
                                tag="osb", bufs=3)
                        ot = osb[(p, tt)]
                        copy_engine(ot[:, cc * 512:(cc + 1) * 512], o_ps[:])
                        if cc == C // 512 - 1:
                            for half in range(2):
                                nc.sync.dma_start(
                                    out[i0 + tt * 128 + half * 64:
                                        i0 + tt * 128 + (half + 1) * 64, :],
                                    ot[half * 64:(half + 1) * 64, :])
                    return go

                # filler queue: independent PE work dripped into the
                # attention stream (keeps the PE gapless while ACT exps)
                fillers = [make_v(tt, "aux", 1) for tt in range(8, 16)]

                for p in (0, 1):
                    i0 = p * PAIR
                    njt = (i0 + PAIR) // 128
                    jlastA = (i0 + 512) // 128 - 1
                    yn = [apool.tile([128, PAIR], BF16, name=f"yn{k}",
                                     tag=f"yn{k}", bufs=2)
                          for k in range(V // 128)]
                    yn_of[p] = yn
                    for h in range(HPC):
                        qrow = (h % 2) * D
                        qtile = qk_t[h // 2]
                        ktile = qk_t[2 + h // 2]
                        y_psA = ps2.tile([128, 512], F32, name="y_psA",
                                         tag="y_ps", bufs=3)
                        y_psB = ps2.tile([128, 512], F32, name="y_psB",
                                         tag="y_ps", bufs=3)
                        def issue_pv(jt, pTt):
                            dlt = max(0, jt * 128 - i0)
                            if dlt < 512:
                                nc.tensor.matmul(
                                    y_psA[:, dlt:512],
                                    v_t[jt][:, h, :],
                                    pTt[:, dlt:512],
                                    start=(jt == 0), stop=(jt == jlastA))
                            loB = max(512, dlt)
                            nc.tensor.matmul(
                                y_psB[:, loB - 512:512],
                                v_t[jt][:, h, :],
                                pTt[:, loB:PAIR],
                                start=(jt == 0), stop=(jt == njt - 1))

                        pv_q = []  # software-pipeline: PV issued 1 block late
                        for jt in range(njt):
                            j0 = jt * 128
                            dlt = max(0, j0 - i0)
                            s_ps = ps2.tile([128, PAIR], F32, name="s_ps",
                                            tag="s_ps", bufs=2)
                            pT = apool.tile([128, PAIR], BF16, name="pT",
                                            tag="pT", bufs=4)
                            diag = j0 >= i0
                            for sub in range(2):
                                lo = max(0, dlt - sub * 512)
                                if lo >= 512:
                                    continue
                                g0 = i0 + sub * 512
                                nc.tensor.matmul(
                                    s_ps[:, sub * 512 + lo:(sub + 1) * 512],
                                    ktile[qrow:qrow + D, j0:j0 + 128],
                                    qtile[qrow:qrow + D, g0 + lo:g0 + 512],
                                    start=True, stop=True)
                            nc.scalar.activation(
                                pT[:, dlt:PAIR], s_ps[:, dlt:PAIR], AF.Exp,
                                scale=float(1.0 / np.sqrt(D)))
                            if diag:
                                # zero the invalid (key > query) half of the
                                # mixed diagonal block on DVE
                                nc.vector.tensor_mul(
                                    pT[:, dlt:dlt + 128],
                                    pT[:, dlt:dlt + 128], tri01[:])
                            pv_q.append((jt, pT))
                            if len(pv_q) > 1:
                                issue_pv(*pv_q.pop(0))
                            # interleave one filler unit into the PE stream
                            if p == 0 and h == 0:
                                if jt < 6:
                                    make_v(jt + 2, "aux", 1)(ps2)
                            elif fillers and (
                                (p == 0 and jt % 3 == 0)
                                or (p == 1 and jt % 4 == 0)
                            ):
                                fillers.pop(0)(ps2)
                        while pv_q:
                            issue_pv(*pv_q.pop(0))
                        # normalize: psum rows 0..63 all hold l (ones cols of
                        # v tile); reciprocal directly on [64, N]
                        rec = apool.tile([D, PAIR], F32, name="rec",
                                         tag="rec", bufs=2)
                        nc.vector.reciprocal_approx_fast(
                            rec[:, 0:512], y_psA[0:D, :])
                        nc.vector.reciprocal_approx_fast(
                            rec[:, 512:PAIR], y_psB[0:D, :])
                        nc.vector.tensor_mul(
                            yn[h // 2][qrow:qrow + D, 0:512],
                            y_psA[D:2 * D, :], rec[:, 0:512])
                        nc.vector.tensor_mul(
                            yn[h // 2][qrow:qrow + D, 512:PAIR],
                            y_psB[D:2 * D, :], rec[:, 512:PAIR])
                    if p == 0:
                        # projection of pair 0 fills pair-1 blocks
                        fillers.extend(
                            proj_mm(0, tt, cc, nc.vector.tensor_copy,
                                    "aux", 1)
                            for tt in range(PAIR // 128)
                            for cc in range(C // 512))
                # drain leftovers, then tail: projection of pair 1 with
                # copies split ACT/DVE and a 3-deep psum ring
                while fillers:
                    fillers.pop(0)(ps2)
                for tt in range(PAIR // 128):
                    for cc in range(C // 512):
                        eng = (nc.scalar.copy if (tt + cc) % 2 == 0
                               else nc.vector.tensor_copy)
                        proj_mm(1, tt, cc, eng, "y_ps", 3)(ps2)
    nc.compile()
    return nc


def _get_nc():
    if "nc" not in _cache:
        _cache["nc"] = _build()
    return _cache["nc"]


def kernel(x, W_attn, b_attn, W_proj, b_proj):
    x = np.asarray(x, dtype=np.float32)
    W_attn = np.asarray(W_attn, dtype=np.float32)
    b_attn = np.asarray(b_attn, dtype=np.float32)
    W_proj = np.asarray(W_proj, dtype=np.float32)
    b_proj = np.asarray(b_proj, dtype=np.float32)

    nc = _get_nc()
    in_maps = []
    for c in range(8):
        b, g = c // 4, c % 4
        in_maps.append({
            "xT": np.ascontiguousarray(x[b].T).astype(ml_dtypes.bfloat16),
            "w_qk": np.ascontiguousarray(
                np.concatenate([W_attn[:, g * V:(g + 1) * V],
                                W_attn[:, C + g * V:C + (g + 1) * V]], axis=1))
                .astype(ml_dtypes.bfloat16),
            "b_qk": np.ascontiguousarray(
                np.concatenate([b_attn[g * V:(g + 1) * V],
                                b_attn[C + g * V:C + (g + 1) * V]])
                .reshape(QK, 1)),
            "w_v": np.ascontiguousarray(W_attn[:, 2 * C + g * V:2 * C + (g + 1) * V])
                .astype(ml_dtypes.bfloat16),
            "b_v": np.ascontiguousarray(b_attn[2 * C + g * V:2 * C + (g + 1) * V]
                                        .reshape(1, V)),
            "w_pr": np.ascontiguousarray(W_proj[g * V:(g + 1) * V, :])
                .astype(ml_dtypes.bfloat16),
        })

    trace = os.environ.get("KTRACE") == "1"
    res = run_bass_kernel_spmd(nc, in_maps, core_ids=list(range(8)),
                               trace=trace)
    _cache["last_exec_ns"] = res.exec_time_ns
    _cache["last_result"] = res

    out = np.zeros((B, T, C), dtype=np.float32)
    for c in range(8):
        out[c // 4] += np.asarray(res.results[c]["out"], dtype=np.float32)
    out += b_proj[None, None, :]
    return out

